# revision 1
# baseline (speedup 1.0000x reference)
"""Trainium2 Bass kernel for a GPT-style transformer block (B=2,T=2048,C=768,H=12).

Sharding: 8 cores; core c handles batch b=c//4, query block qo=(c%4)*512.
Each core gets its batch's x feature-major [C,T], rolled so its 512 query
tokens are columns 0:512.  K/V are computed for all 2048 keys (duplicated
across the 4 cores of a batch); Q/attention/MLP only for the 512 queries.

Perf structure (the PE only reaches 2.4GHz when its instruction stream is
gapless, else it runs at the 1.2GHz mid p-state):
 - x is never centered and never squared elementwise.  Sum(x^2) comes from
   PE Gram matmuls (x as both operands) with the diagonal extracted by a
   masked tensor_tensor_reduce; the -mu correction folds into every
   projection as one rank-1 PE accumulation (host-precomputed col sums).
 - Only K chunk oc=0 is computed up front; chunks oc=1..5 are spread through
   the attention inner loop (~1.5 matmuls/iter) as PE filler while the
   softmax exp runs on the Act engine.
 - Scores for iteration i+1 are emitted before AV of iteration i (software
   pipelining) so the PE never waits on the exp.
 - Denominators: augmented-V ones column (row 0) -> row 0 of the AV psum;
   broadcast via a rank-1 PE matmul, then a 65-partition DVE reciprocal.
 - MLP: bf16 weights, rank-1 LN2 mean fold + istd multiply at PSUM
   copyback, gelu delayed one kc so PE streams FC/proj gaplessly.
"""
import sys

sys.path.insert(0, "/opt/trn_rl_repo")

import numpy as np
import ml_dtypes

import concourse.bass as bass
import concourse.tile as tile
from concourse import bacc, mybir
from concourse.bass_utils import run_bass_kernel_spmd

F32 = mybir.dt.float32
F32R = mybir.dt.float32r
BF16 = mybir.dt.bfloat16
AF = mybir.ActivationFunctionType
ALU = mybir.AluOpType

B, T, C, H = 2, 2048, 768, 12
HD = C // H             # 64
C4 = 4 * C              # 3072
EPS = 1e-5
NCORES = 8
TQ = (B * T) // NCORES  # 512
PC = C // 128           # 6
PC4 = C4 // 128         # 24
NT4 = T // 512          # 4
NSC = T // 128          # 16
NBIAS = (5 * C + C4) // 128  # 54


def _build(has_qkv_bias, has_o_bias, has_proj_bias, has_fc_bias, has_mask, reps=1):
    has_bias_any = has_qkv_bias or has_o_bias or has_proj_bias or has_fc_bias
    nc = bacc.Bacc()

    x_d = nc.dram_tensor("x_fm", [C, T], F32, kind="ExternalInput")
    xb_d = nc.dram_tensor("x_bf", [C, T], BF16, kind="ExternalInput")
    wq_d = nc.dram_tensor("wq", [C, C], BF16, kind="ExternalInput")
    wk_d = nc.dram_tensor("wk", [C, C], BF16, kind="ExternalInput")
    wv_d = nc.dram_tensor("wv", [C, C], BF16, kind="ExternalInput")
    wo_d = nc.dram_tensor("wo", [128, PC, C], BF16, kind="ExternalInput")
    wfc_d = nc.dram_tensor("wfc", [PC4, 128, PC, 128], BF16, kind="ExternalInput")
    wproj_d = nc.dram_tensor("wproj", [C4, C], BF16, kind="ExternalInput")
    wsum3_d = nc.dram_tensor("wsums3", [1, 3 * C], BF16, kind="ExternalInput")
    wsumfc_d = nc.dram_tensor("wsumsfc", [1, C4], BF16, kind="ExternalInput")
    ident_d = nc.dram_tensor("ident", [128, 128], BF16, kind="ExternalInput")
    bias_d = nc.dram_tensor("biases", [128, NBIAS], F32, kind="ExternalInput")
    brow_d = nc.dram_tensor("bias_rows", [1, 3 * C], BF16, kind="ExternalInput")
    mask_d = nc.dram_tensor("maskb", [128, NSC], F32, kind="ExternalInput")
    out_d = nc.dram_tensor("out_fm", [C, TQ], F32, kind="ExternalOutput")

    x_pot = x_d.rearrange("(o p) t -> p o t", p=128)
    xb_pot = xb_d.rearrange("(o p) t -> p o t", p=128)

    with tile.TileContext(nc) as tc:
      for _rep in range(reps):
        with tc.tile_pool(name=f"const{_rep}", bufs=1) as const, \
             tc.tile_pool(name=f"persist{_rep}", bufs=1) as persist:

            # ---------------- constants ----------------
            ones_f = const.tile([128, 1], F32)
            nc.vector.memset(ones_f[:], 1.0)
            ones_col_b = const.tile([128, 1], BF16)
            nc.vector.memset(ones_col_b[:], 1.0)
            ones_col_r = const.tile([128, 1], F32R)
            nc.vector.tensor_copy(ones_col_r[:], ones_f[:])
            onesr_f = const.tile([1, 128], F32)
            nc.vector.memset(onesr_f[:], 1.0)
            ones_row = const.tile([1, 128], F32R)
            nc.vector.tensor_copy(ones_row[:], onesr_f[:])
            ones_row_b = const.tile([1, 128], BF16)
            nc.vector.memset(ones_row_b[:], 1.0)
            ident_sb = const.tile([128, 128], BF16)
            nc.sync.dma_start(ident_sb[:], ident_d[:, :])
            ones_mat = const.tile([128, HD + 1], BF16)
            nc.vector.memset(ones_mat[:], 1.0)
            wsum_sb = const.tile([1, 3 * C], BF16)
            nc.sync.dma_start(wsum_sb[:], wsum3_d[:, :])

            if has_bias_any:
                bias_sb = const.tile([128, NBIAS], F32)
                nc.sync.dma_start(bias_sb[:], bias_d[:, :])
            if has_mask:
                mask_sb = const.tile([128, NSC], F32)
                nc.sync.dma_start(mask_sb[:], mask_d[:, :])
            if has_qkv_bias:
                brow_sb = const.tile([1, 3 * C], BF16)
                nc.sync.dma_start(brow_sb[:], brow_d[:, :])

            x_q = persist.tile([128, PC, TQ], F32)   # exact residual copy
            nc.sync.dma_start(x_q[:], x_pot[:, :, 0:TQ])

            wo_sb = persist.tile([128, PC, C], BF16)
            nc.sync.dma_start(wo_sb[:], wo_d[:, :, :])
            x2 = persist.tile([128, PC, TQ], F32R)
            with tc.tile_pool(name=f"ypool{_rep}", bufs=1) as ypool:
              y_sb = ypool.tile([HD + 1, H, TQ], BF16)
              y_nm2 = ypool.tile([128, PC, TQ], BF16)
              with tc.tile_pool(name=f"attp{_rep}", bufs=1) as attp:
                q_pad = attp.tile([128, H, TQ], BF16)
                k_bf = attp.tile([128, PC, T], BF16)
                vt_aug = attp.tile([128, NSC, H * (HD + 1)], BF16)
                x_bf = attp.tile([128, PC, T], BF16)
                nc.sync.dma_start(x_bf[:], xb_pot[:, :, :])
                istd_b = attp.tile([128, T], BF16)
                istd_col = attp.tile([128, NSC], F32)
                risd_r = (attp.tile([1, T], BF16)    # sqrt(var+eps) (bias path)
                          if has_qkv_bias else None)
                wk_sb = attp.tile([128, PC, C], BF16)
                nc.sync.dma_start(wk_sb[:], wk_d.rearrange("(o p) m -> p o m", p=128))

                # ======== phases A+B: LN1 stats + V/Q + K(oc=0) ========
                with tc.tile_pool(name=f"rtmp{_rep}", bufs=2) as rtmp, \
                     tc.tile_pool(name=f"gsc{_rep}", bufs=2) as gsc, \
                     tc.tile_pool(name=f"wcyc{_rep}", bufs=2) as wcyc, \
                     tc.tile_pool(name=f"st_ps{_rep}", bufs=1, space="PSUM") as st_ps, \
                     tc.tile_pool(name=f"p1_ps{_rep}", bufs=2, space="PSUM") as p1_ps, \
                     tc.tile_pool(name=f"g_ps{_rep}", bufs=2, space="PSUM") as g_ps, \
                     tc.tile_pool(name=f"vq_ps{_rep}", bufs=2, space="PSUM") as vq_ps:

                    wv_sb = wcyc.tile([128, PC, C], BF16, tag="w")
                    nc.sync.dma_start(wv_sb[:], wv_d.rearrange("(o p) m -> p o m", p=128))

                    for t4 in range(NT4):
                        sl = slice(t4 * 512, (t4 + 1) * 512)
                        p1 = p1_ps.tile([1, 512], F32, tag="p1")
                        for j in range(PC):
                            nc.tensor.matmul(p1[:], ones_col_b[:], x_bf[:, j, sl],
                                             start=(j == 0), stop=(j == PC - 1))
                        negmu_c = rtmp.tile([1, 512], BF16, tag="rtb")
                        nc.vector.tensor_scalar_mul(negmu_c[:], p1[:], -1.0 / C)
                        nm_ps = st_ps.tile([128, 512], F32, tag="nm")
                        nc.tensor.matmul(nm_ps[:], ones_row_b[:], negmu_c[:],
                                         start=True, stop=True)
                        nm_sb = gsc.tile([128, 512], BF16, tag="nmsb")
                        nc.vector.tensor_copy(nm_sb[:], nm_ps[:])
                        for j in range(PC):
                            eng = nc.vector if j < 3 else nc.gpsimd
                            eng.tensor_tensor(x_bf[:, j, sl], x_bf[:, j, sl],
                                              nm_sb[:], ALU.add)
                        p2 = p1_ps.tile([1, 512], F32, tag="p2")
                        for j in range(PC):
                            xsq = gsc.tile([128, 512], BF16, tag="xsq")
                            if j < 3:
                                nc.scalar.activation(xsq[:], x_bf[:, j, sl], AF.Square)
                            else:
                                eng = nc.vector if j < 5 else nc.gpsimd
                                eng.tensor_tensor(xsq[:], x_bf[:, j, sl], x_bf[:, j, sl],
                                                  ALU.mult)
                            nc.tensor.matmul(p2[:], ones_col_b[:], xsq[:],
                                             start=(j == 0), stop=(j == PC - 1))
                        var_c = rtmp.tile([1, 512], F32, tag="rt")
                        nc.vector.tensor_scalar(var_c[:], p2[:], 1.0 / C, EPS,
                                                ALU.mult, ALU.add)
                        lnv_c = rtmp.tile([1, 512], F32, tag="rt")
                        nc.scalar.activation(lnv_c[:], var_c[:], AF.Ln)
                        istd_c = rtmp.tile([1, 512], F32, tag="rt")
                        nc.scalar.activation(istd_c[:], lnv_c[:], AF.Exp, scale=-0.5)
                        istd_cb = rtmp.tile([1, 512], BF16, tag="rtb")
                        nc.vector.tensor_copy(istd_cb[:], istd_c[:])
                        if has_qkv_bias:
                            nc.scalar.activation(risd_r[:, sl], lnv_c[:], AF.Exp, scale=0.5)
                        bp = st_ps.tile([128, 512], F32, tag="bp")
                        nc.tensor.matmul(bp[:], ones_row_b[:], istd_cb[:],
                                         start=True, stop=True)
                        nc.scalar.activation(istd_b[:, sl], bp[:], AF.Copy)
                        for o in range(4):
                            nc.sync.dma_start(istd_col[:, t4 * 4 + o:t4 * 4 + o + 1],
                                              istd_c[0:1, o * 128:(o + 1) * 128])

                    # ---- V (all keys, token-major, ones col last) ----
                    for sc in range(NSC):
                        ssl = slice(sc * 128, (sc + 1) * 128)
                        nc.gpsimd.memset(
                            vt_aug[:, sc, :].rearrange("p (h e) -> p h e", e=HD + 1)[:, :, HD:HD + 1],
                            1.0)
                        nc.gpsimd.memset(
                            vt_aug[:, sc, :].rearrange("p (h e) -> p h e", e=HD + 1)[:, :, 0:1],
                            1.0)
                        for half in range(2):
                            hsl = slice(half * 384, (half + 1) * 384)
                            csl = slice(2 * C + half * 384, 2 * C + (half + 1) * 384)
                            vp = vq_ps.tile([128, 512], F32, tag="pp", name="vp")[:, 0:384]
                            for j in range(PC):
                                nc.tensor.matmul(vp[:], x_bf[:, j, ssl], wv_sb[:, j, hsl],
                                                 start=(j == 0),
                                                 stop=(j == PC - 1 and not has_qkv_bias))
                            if has_qkv_bias:
                                nc.tensor.matmul(vp[:], risd_r[:, ssl], brow_sb[:, csl],
                                                 start=False, stop=True)
                            dst = vt_aug[:, sc, :].rearrange("p (h e) -> p h e", e=HD + 1)[
                                :, half * 6:(half + 1) * 6, 0:HD]
                            if half == 0:
                                nc.scalar.activation(
                                    dst, vp[:].rearrange("p (h e) -> p h e", e=HD),
                                    AF.Copy, scale=istd_col[:, sc:sc + 1])
                            else:
                                nc.vector.tensor_scalar(
                                    dst, vp[:].rearrange("p (h e) -> p h e", e=HD),
                                    istd_col[:, sc:sc + 1], None, ALU.mult)

                    # ---- Q (queries only) ----
                    wq_sb = wcyc.tile([128, PC, C], BF16, tag="w")
                    nc.sync.dma_start(wq_sb[:], wq_d.rearrange("(o p) m -> p o m", p=128))
                    for h in range(H):
                        base = 64 * (h & 1)
                        nc.gpsimd.memset(q_pad[64 - base:128 - base, h, :], 0.0)
                    for oc in range(PC):
                        osl = slice(oc * 128, (oc + 1) * 128)
                        qp = vq_ps.tile([128, 512], F32, tag="pp", name="qp")
                        for j in range(PC):
                            nc.tensor.matmul(qp[:], wq_sb[:, j, osl],
                                             x_bf[:, j, 0:TQ], start=(j == 0),
                                             stop=(j == PC - 1 and not has_qkv_bias))
                        if has_qkv_bias:
                            nc.tensor.matmul(qp[:], brow_sb[:, osl],
                                             risd_r[:, 0:TQ], start=False, stop=True)
                        nc.vector.tensor_tensor(q_pad[0:64, 2 * oc, :], qp[0:64, :],
                                                istd_b[0:64, 0:TQ], ALU.mult)
                        nc.vector.tensor_tensor(q_pad[64:128, 2 * oc + 1, :], qp[64:128, :],
                                                istd_b[64:128, 0:TQ], ALU.mult)
                    # ---- K chunk oc=0 only (rest run inside attention) ----
                    for t4 in range(NT4):
                        sl = slice(t4 * 512, (t4 + 1) * 512)
                        kp = vq_ps.tile([128, 512], F32, tag="pp", name="kp")
                        for j in range(PC):
                            nc.tensor.matmul(kp[:], wk_sb[:, j, 0:128],
                                             x_bf[:, j, sl], start=(j == 0),
                                             stop=(j == PC - 1 and not has_qkv_bias))
                        if has_qkv_bias:
                            nc.tensor.matmul(kp[:], brow_sb[:, C:C + 128],
                                             risd_r[:, sl], start=False, stop=True)
                        nc.vector.tensor_tensor(k_bf[:, 0, sl], kp[:], istd_b[:, sl],
                                                ALU.mult)

                # ============ phase C: attention (K oc=1..5 interleaved) ====
                if True:
                    with tc.tile_pool(name=f"sc_ps{_rep}", bufs=2, space="PSUM") as sc_ps, \
                         tc.tile_pool(name=f"y_psp{_rep}", bufs=2, space="PSUM") as y_psp, \
                         tc.tile_pool(name=f"rp_ps{_rep}", bufs=1, space="PSUM") as rp_ps, \
                         tc.tile_pool(name=f"kp_ps{_rep}", bufs=1, space="PSUM") as kp_ps, \
                         tc.tile_pool(name=f"attb{_rep}", bufs=3) as attb, \
                         tc.tile_pool(name=f"recb{_rep}", bufs=2) as recb:

                        def k_chunk_gen(oc):
                            # yields after each PE matmul; copybacks on DVE
                            osl = slice(oc * 128, (oc + 1) * 128)
                            for t4 in range(NT4):
                                sl = slice(t4 * 512, (t4 + 1) * 512)
                                kp = kp_ps.tile([128, 512], F32, tag="kp")
                                for j in range(PC):
                                    nc.tensor.matmul(
                                        kp[:], wk_sb[:, j, osl],
                                        x_bf[:, j, sl], start=(j == 0),
                                        stop=(j == PC - 1 and not has_qkv_bias))
                                    if j < PC - 1:
                                        yield
                                if has_qkv_bias:
                                    nc.tensor.matmul(
                                        kp[:], brow_sb[:, C + oc * 128:C + (oc + 1) * 128],
                                        risd_r[:, sl], start=False, stop=True)
                                nc.vector.tensor_tensor(k_bf[:, oc, sl], kp[:],
                                                        istd_b[:, sl], ALU.mult)
                                yield

                        def make_tail(h, yp):
                            def tail():
                                nc.vector.tensor_copy(y_sb[:, h, :], yp[:])
                                rp = rp_ps.tile([HD + 1, TQ], F32, tag="rp")
                                nc.tensor.matmul(rp[:], ones_mat[64:65, 0:HD + 1],
                                                 y_sb[HD:HD + 1, h, :],
                                                 start=True, stop=True)
                                rec = recb.tile([HD + 1, TQ], F32, tag="rec")
                                nc.vector.reciprocal(rec[:], rp[:])
                                if h % 2 == 0:
                                    nc.gpsimd.tensor_tensor(y_nm2[0:HD, h // 2, :],
                                                            y_sb[0:HD, h, :],
                                                            rec[0:HD, :], ALU.mult)
                                else:
                                    ytmp = recb.tile([HD, TQ], BF16, tag="ytmp")
                                    nc.gpsimd.tensor_tensor(ytmp[:], y_sb[0:HD, h, :],
                                                            rec[0:HD, :], ALU.mult)
                                    nc.sync.dma_start(y_nm2[HD:128, h // 2, :], ytmp[:])
                            return tail

                        kgen = None
                        pending_tail = None
                        for h in range(H):
                            base = 64 * (h & 1)
                            ch = h // 2
                            if h < 10 and h % 2 == 0:
                                kgen = k_chunk_gen(1 + h // 2)
                            yp = y_psp.tile([HD + 1, TQ], F32, tag="yp")
                            prev_av = None
                            for scp in range(NSC // 2):
                                sp = sc_ps.tile([128, 2, 512], F32, tag="sp")
                                for i in range(2):
                                    sc = 2 * scp + i
                                    nc.tensor.matmul(
                                        sp[:, i, :],
                                        k_bf[:, ch, sc * 128:(sc + 1) * 128],
                                        q_pad[:, h, :],
                                        start=True, stop=True)
                                att = attb.tile([128, 2, 512], BF16, tag="att")
                                if has_mask:
                                    for i in range(2):
                                        sc = 2 * scp + i
                                        nc.scalar.activation(att[:, i, :], sp[:, i, :], AF.Exp,
                                                             bias=mask_sb[:, sc:sc + 1])
                                else:
                                    nc.scalar.activation(att[:], sp[:], AF.Exp)
                                if prev_av is not None:
                                    prev_av()
                                if pending_tail is not None:
                                    pending_tail()
                                    pending_tail = None
                                if kgen is not None:
                                    for _ in range(2 if scp % 2 == 0 else 1):
                                        if next(kgen, "end") == "end":
                                            kgen = None
                                            break

                                def av(att=att, scp=scp):
                                    for i in range(2):
                                        sc = 2 * scp + i
                                        nc.tensor.matmul(yp[:], vt_aug[:, sc, 65 * h:65 * h + 65],
                                                         att[:, i, :],
                                                         start=(sc == 0), stop=(sc == NSC - 1))
                                prev_av = av
                            prev_av()
                            pending_tail = make_tail(h, yp)
                        pending_tail()

              # ---- Wo (pairs 0-4 first so the last head's tail overlaps) ----
              with tc.tile_pool(name=f"dtmp{_rep}", bufs=2) as dtmp, \
                   tc.tile_pool(name=f"drow{_rep}", bufs=1) as drow:
                xc_bf = drow.tile([128, PC, TQ], BF16)
                x2b = drow.tile([128, PC, TQ], BF16)
                with tc.tile_pool(name=f"wo_ps{_rep}", bufs=1, space="PSUM") as wo_ps, \
                     tc.tile_pool(name=f"xsqp{_rep}", bufs=2) as xsqp, \
                     tc.tile_pool(name=f"d_ps{_rep}", bufs=1, space="PSUM") as d_ps:
                  p1 = d_ps.tile([1, TQ], F32, tag="p1")
                  p2 = d_ps.tile([1, TQ], F32, tag="p2")
                  wops = []
                  for oc in range(PC):
                      op = wo_ps.tile([128, TQ], F32, tag=f"op{oc}", name=f"op{oc}")
                      for hp in range(PC - 1):
                          nc.tensor.matmul(op[:], wo_sb[:, hp, oc * 128:(oc + 1) * 128],
                                           y_nm2[:, hp, :], start=(hp == 0), stop=False)
                      wops.append(op)
                  for oc in range(PC):
                      nc.tensor.matmul(wops[oc][:], wo_sb[:, PC - 1, oc * 128:(oc + 1) * 128],
                                       y_nm2[:, PC - 1, :], start=False, stop=True)
                  for oc in range(PC):
                      op = wops[oc]
                      if has_o_bias:
                          nc.scalar.activation(op[:], op[:], AF.Identity,
                                               bias=bias_sb[:, 3 * PC + oc:3 * PC + oc + 1])
                      nc.vector.tensor_tensor(x2[:, oc, :], op[:], x_q[:, oc, :],
                                              ALU.add)
                      nc.gpsimd.tensor_copy(x2b[:, oc, :], x2[:, oc, :])
                      xsqa = xsqp.tile([128, TQ], BF16, tag="xsqa")
                      nc.gpsimd.tensor_tensor(xsqa[:], x2b[:, oc, :], x2b[:, oc, :],
                                              ALU.mult)
                      nc.tensor.matmul(p1[:], ones_col_r[:], x2[:, oc, :],
                                       start=(oc == 0), stop=(oc == PC - 1))
                      nc.tensor.matmul(p2[:], ones_col_b[:], xsqa[:],
                                       start=(oc == 0), stop=(oc == PC - 1))

                  # LN2 scalars that read p1/p2 (before d_ps closes)
                  mean2 = dtmp.tile([1, TQ], F32, tag="dt")
                  nc.vector.tensor_scalar_mul(mean2[:], p1[:], 1.0 / C)
                  msq2 = dtmp.tile([1, TQ], F32, tag="dt")
                  nc.vector.tensor_tensor(msq2[:], mean2[:], mean2[:], ALU.mult)
                  var2 = dtmp.tile([1, TQ], F32, tag="dt")
                  nc.vector.tensor_scalar(var2[:], p2[:], 1.0 / C, EPS,
                                          ALU.mult, ALU.add)
                  nc.vector.tensor_sub(var2[:], var2[:], msq2[:])
                  negmu2_r = drow.tile([1, TQ], F32R)
                  nc.vector.tensor_scalar_mul(negmu2_r[:], mean2[:], -1.0)

                lnv2 = dtmp.tile([1, TQ], F32, tag="dt")
                nc.scalar.activation(lnv2[:], var2[:], AF.Ln)
                istd2 = dtmp.tile([1, TQ], F32, tag="dt")
                nc.scalar.activation(istd2[:], lnv2[:], AF.Exp, scale=-0.5)
                istd2_r = drow.tile([1, TQ], F32R)
                nc.vector.tensor_copy(istd2_r[:], istd2[:])
                istd2_b = drow.tile([128, TQ], BF16)
                nm2_b = drow.tile([128, TQ], BF16)
                with tc.tile_pool(name=f"b2_ps{_rep}", bufs=1, space="PSUM") as b2_ps:
                    bp2 = b2_ps.tile([128, TQ], F32, tag="bp2")
                    nc.tensor.matmul(bp2[:], ones_row[:], istd2_r[:],
                                     start=True, stop=True)
                    nc.scalar.activation(istd2_b[:], bp2[:], AF.Copy)
                    nm2_ps = b2_ps.tile([128, TQ], F32, tag="nm2")
                    nc.tensor.matmul(nm2_ps[:], ones_row[:], negmu2_r[:],
                                     start=True, stop=True)
                    nc.scalar.activation(nm2_b[:], nm2_ps[:], AF.Copy)
                    # center + scale (all bf16 -> DVE fast mode)
                    for j in range(PC):
                        xct = dtmp.tile([128, TQ], BF16, tag="xct")
                        nc.vector.tensor_tensor(xct[:], x2b[:, j, :], nm2_b[:], ALU.add)
                        nc.vector.tensor_tensor(xc_bf[:, j, :], xct[:], istd2_b[:],
                                                ALU.mult)

                # ============ phase E: MLP ============
                out_sb = persist.tile([128, PC, TQ], F32)
                with tc.tile_pool(name=f"pr_ps{_rep}", bufs=1, space="PSUM") as pr_ps, \
                     tc.tile_pool(name=f"fc_ps{_rep}", bufs=2, space="PSUM") as fc_ps, \
                     tc.tile_pool(name=f"h_sb{_rep}", bufs=2) as h_sb, \
                     tc.tile_pool(name=f"w_sb2{_rep}", bufs=3) as w_sb2:
                      prs = [pr_ps.tile([128, TQ], F32, tag=f"pr{i}", name=f"pr{i}")
                             for i in range(PC)]
                      hcs = {}
                      for kc in range(PC4):
                          wfcc = w_sb2.tile([128, PC, 128], BF16, tag="wfcc")
                          nc.sync.dma_start(wfcc[:], wfc_d[kc])
                          fp = fc_ps.tile([128, TQ], F32, tag="fp")
                          for j in range(PC):
                              nc.tensor.matmul(fp[:], wfcc[:, j, :], xc_bf[:, j, :],
                                               start=(j == 0), stop=(j == PC - 1))
                          hc = h_sb.tile([128, TQ], BF16, tag="hc")
                          if has_fc_bias:
                              nc.scalar.activation(hc[:], fp[:], AF.Gelu,
                                                   bias=bias_sb[:, 5 * PC + kc:5 * PC + kc + 1])
                          else:
                              nc.scalar.activation(hc[:], fp[:], AF.Gelu)
                          wpc = w_sb2.tile([128, C], BF16, tag="wpc")
                          nc.sync.dma_start(wpc[:], wproj_d[kc * 128:(kc + 1) * 128, :])
                          hcs[kc] = (hc, wpc)
                          # delay proj by one kc so gelu overlaps the next FC
                          if kc >= 1:
                              hcp, wpcp = hcs.pop(kc - 1)
                              for oc in range(PC):
                                  nc.tensor.matmul(prs[oc][:],
                                                   wpcp[:, oc * 128:(oc + 1) * 128],
                                                   hcp[:], start=(kc - 1 == 0), stop=False)
                      hcp, wpcp = hcs.pop(PC4 - 1)
                      for oc in range(PC):
                          nc.tensor.matmul(prs[oc][:], wpcp[:, oc * 128:(oc + 1) * 128],
                                           hcp[:], start=False, stop=True)
                      for oc in range(PC):
                          if has_proj_bias:
                              nc.scalar.activation(prs[oc][:], prs[oc][:], AF.Identity,
                                                   bias=bias_sb[:, 4 * PC + oc:4 * PC + oc + 1])
                          nc.vector.tensor_tensor(out_sb[:, oc, :], prs[oc][:],
                                                  x2[:, oc, :], ALU.add)
            nc.sync.dma_start(out_d.rearrange("(o p) t -> p o t", p=128), out_sb[:])

    nc.compile()
    return nc


_CACHE = {}


def _get_program(flags, reps=1):
    key = (flags, reps)
    if key not in _CACHE:
        _CACHE[key] = _build(*flags, reps=reps)
    return _CACHE[key]


def kernel(**inputs) -> np.ndarray:
    x = np.asarray(inputs["x"], dtype=np.float32)
    padding_mask = np.asarray(inputs["padding_mask"])
    ln1_s = np.asarray(inputs["ln1_scale"], dtype=np.float32)
    ln1_b = np.asarray(inputs["ln1_bias"], dtype=np.float32)
    ln2_s = np.asarray(inputs["ln2_scale"], dtype=np.float32)
    ln2_b = np.asarray(inputs["ln2_bias"], dtype=np.float32)
    Wq = np.asarray(inputs["Wq"], dtype=np.float32)
    Wk = np.asarray(inputs["Wk"], dtype=np.float32)
    Wv = np.asarray(inputs["Wv"], dtype=np.float32)
    bq = np.asarray(inputs["bq"], dtype=np.float32)
    bk = np.asarray(inputs["bk"], dtype=np.float32)
    bv = np.asarray(inputs["bv"], dtype=np.float32)
    Wo = np.asarray(inputs["Wo"], dtype=np.float32)
    bo = np.asarray(inputs["bo"], dtype=np.float32)
    Wfc = np.asarray(inputs["Wfc"], dtype=np.float32)
    bfc = np.asarray(inputs["bfc"], dtype=np.float32)
    Wproj = np.asarray(inputs["Wproj"], dtype=np.float32)
    bproj = np.asarray(inputs["bproj"], dtype=np.float32)

    sc_q = 1.0 / np.sqrt(HD)
    Wq_f = Wq.transpose(1, 0, 2).reshape(C, C)
    Wk_f = Wk.transpose(1, 0, 2).reshape(C, C)
    Wv_f = Wv.transpose(1, 0, 2).reshape(C, C)
    wq_eff = (ln1_s[:, None] * Wq_f * sc_q).astype(ml_dtypes.bfloat16)
    wk_eff = (ln1_s[:, None] * Wk_f).astype(ml_dtypes.bfloat16)
    wv_eff = (ln1_s[:, None] * Wv_f).astype(ml_dtypes.bfloat16)
    bq_eff = (ln1_b @ Wq_f) * sc_q + bq.reshape(C) * sc_q
    bk_eff = ln1_b @ Wk_f + bk.reshape(C)
    bv_eff = ln1_b @ Wv_f + bv.reshape(C)
    wfc_eff = (ln2_s[:, None] * Wfc).astype(ml_dtypes.bfloat16)
    bfc_eff = ln2_b @ Wfc + bfc
    wfc_pre = np.ascontiguousarray(
        wfc_eff.reshape(PC, 128, PC4, 128).transpose(2, 1, 0, 3))
    wproj_b = Wproj.astype(ml_dtypes.bfloat16)
    wo_pre = np.ascontiguousarray(
        Wo.reshape(PC, 2, HD, C).transpose(1, 2, 0, 3).reshape(128, PC, C)
    ).astype(ml_dtypes.bfloat16)

    # column sums of the weights as the PE sees them (bf16 for q/k/v)
    wsums3 = np.concatenate(
        [wq_eff.astype(np.float32).sum(0), wk_eff.astype(np.float32).sum(0),
         wv_eff.astype(np.float32).sum(0)]).astype(ml_dtypes.bfloat16)[None, :]
    wsumsfc = wfc_eff.astype(np.float32).sum(0).astype(ml_dtypes.bfloat16)[None, :]
    ident = np.eye(128, dtype=np.float32).astype(ml_dtypes.bfloat16)

    biases = np.concatenate([bq_eff, bk_eff, bv_eff, bo, bproj, bfc_eff])
    bias_pre = np.ascontiguousarray(biases.reshape(NBIAS, 128).T).astype(np.float32)
    brows = np.concatenate([bq_eff, bk_eff, bv_eff]).astype(ml_dtypes.bfloat16)[None, :]

    has_qkv_bias = bool(np.abs(np.concatenate([bq_eff, bk_eff, bv_eff])).max() > 0)
    has_o_bias = bool(np.abs(bo).max() > 0)
    has_proj_bias = bool(np.abs(bproj).max() > 0)
    has_fc_bias = bool(np.abs(bfc_eff).max() > 0)
    has_mask = bool(padding_mask.any())

    nc = _get_program((has_qkv_bias, has_o_bias, has_proj_bias, has_fc_bias, has_mask))

    shared = {
        "wq": wq_eff, "wk": wk_eff, "wv": wv_eff, "wo": wo_pre,
        "wfc": wfc_pre, "wproj": wproj_b, "wsums3": wsums3, "wsumsfc": wsumsfc,
        "ident": ident, "biases": bias_pre, "bias_rows": brows,
    }
    in_maps = []
    for c in range(NCORES):
        b, qo = c // (NCORES // B), (c % (NCORES // B)) * TQ
        xr = np.roll(x[b], -qo, axis=0)
        x_fm = np.ascontiguousarray(xr.T)
        x_bf = x_fm.astype(ml_dtypes.bfloat16)
        mrow = np.roll(padding_mask[b], -qo)
        maskb = np.ascontiguousarray(
            np.where(mrow, -1e30, 0.0).astype(np.float32).reshape(NSC, 128).T)
        in_maps.append({**shared, "x_fm": x_fm, "x_bf": x_bf, "maskb": maskb})

    res = run_bass_kernel_spmd(nc, in_maps, core_ids=list(range(NCORES)))

    out = np.empty((B, T, C), dtype=np.float32)
    for c in range(NCORES):
        b, qo = c // (NCORES // B), (c % (NCORES // B)) * TQ
        out[b, qo:qo + TQ, :] = res.results[c]["out_fm"].T
    return out

